# revision 27
# baseline (speedup 1.0000x reference)
"""Causal attention (B=4, N=2048, D=1024) on 8 Trainium2 NeuronCores.

Sharding: core 2b+p handles batch b with query tiles {p, p+2, ..., p+14}
(128-row tiles, parity-interleaved).  Every core runs the same program:
8 query slots with key-tile limits (2, 4, ..., 16) — an exactly balanced
causal split.  Per-core masks are passed as input data so the program is
uniform across cores (SPMD).

All matmuls run in float32r (TF32-like, full PE rate at N>=256); fp32
arrays are fed bit-identically into float32r DRAM params (HW rounds at
the PE input).  x is pre-transposed on the host into d-major tile layout
so no on-chip transposes are needed for the projections.

Schedule: Q^T is computed first and spilled to DRAM; then keys are
processed in two halves (V + K^T into SBUF-resident tiles), with
attention slots 0-3 placed between the halves so the scheduler can
overlap early attention with the second half's projections.  Softmax is
single-pass over the full key row (<= 4 PSUM banks) with exp + row-sum
fused on the scalar engine.
"""
import sys

sys.path.insert(0, "/opt/trn_rl_repo")

from contextlib import ExitStack

import numpy as np

import concourse.bass as bass
import concourse.mybir as mybir
import concourse.tile as tile
from concourse import bacc
from concourse.bass_utils import run_bass_kernel_spmd
from concourse.masks import make_identity

B, N, D = 4, 2048, 1024
N_CORES = 8
N_SLOTS = 8          # query tiles per core
N_KTILES = 16        # 128-key tiles per batch
SCALE = 1.0 / 32.0   # 1/sqrt(D)
NEG = -1.0e9

F32 = mybir.dt.float32
F32R = mybir.dt.float32r

_NC_CACHE = {}
TRACE = False
LAST_EXEC_NS = None


def _build_nc():
    nc = bacc.Bacc(None, target_bir_lowering=False, debug=False)

    # x halves pre-transposed on host: [tile, partition(d%128), dchunk, token]
    # x_h = this core's own 8 key tiles (even core: 0-7, odd core: 8-15)
    x_h = nc.declare_dram_parameter("x_h", [8, 128, 8, 128], F32R, isOutput=False)
    x_qt = nc.declare_dram_parameter("x_qt", [N_SLOTS, 128, 8, 128], F32R, isOutput=False)
    # weights host-rearranged: wq/wk [echunk, p(d%128), dchunk, ecol]; wv [eh, p, dchunk, ecol]
    wq = nc.declare_dram_parameter("wq", [8, 128, 8, 128], F32R, isOutput=False)
    wk = nc.declare_dram_parameter("wk", [8, 128, 8, 128], F32R, isOutput=False)
    wv = nc.declare_dram_parameter("wv", [2, 128, 8, 512], F32R, isOutput=False)
    mask_in = nc.declare_dram_parameter("mask", [128, 256], F32, isOutput=False)
    out_q = nc.declare_dram_parameter("out_q", [N_SLOTS, 128, D], F32, isOutput=True)

    # DRAM scratch: Q^T per-slot-contiguous
    qt_spill = nc.dram_tensor("qt_spill", [N_SLOTS, 128, 8, 128], F32R, kind="Internal")

    with tile.TileContext(nc) as tc, ExitStack() as top:
        consts = top.enter_context(tc.tile_pool(name="consts", bufs=1))
        kt_pool = top.enter_context(tc.tile_pool(name="ktp", bufs=1))
        v_pool = top.enter_context(tc.tile_pool(name="vp", bufs=1))
        qt_pool2 = top.enter_context(tc.tile_pool(name="qtl", bufs=2))
        dram = top.enter_context(tc.tile_pool(name="dram", bufs=1, space="DRAM"))
        ps_tr = top.enter_context(tc.tile_pool(name="ps_tr", bufs=2, space="PSUM"))
        ps_o = top.enter_context(tc.tile_pool(name="ps_o", bufs=1, space="PSUM"))

        ident_f = consts.tile([128, 128], F32)
        make_identity(nc, ident_f)
        ident = consts.tile([128, 128], F32R)
        nc.vector.tensor_copy(ident, ident_f)
        mask_sb = consts.tile([128, 256], F32)
        nc.sync.dma_start(out=mask_sb, in_=mask_in[:, :])

        KT = kt_pool.tile([128, 8, N], F32R)      # [p(e%128), echunk, key]
        V = v_pool.tile([128, 12, D], F32R)       # [p(key%128), ktile<12, e]

        # collective bounce: [p, 0:8 = KT-own echunks, 8:16 = V-own tiles, 1024]
        cc_in = dram.tile([128, 16, 1024], F32R)
        cc_out = dram.tile([2, 128, 16, 1024], F32R)

        with ExitStack() as ph12:
            xt_pool = ph12.enter_context(tc.tile_pool(name="xtp", bufs=1))
            wv_pool = ph12.enter_context(tc.tile_pool(name="wvp", bufs=2))
            we_pool = ph12.enter_context(tc.tile_pool(name="wep", bufs=2))
            qst_pool = ph12.enter_context(tc.tile_pool(name="qst", bufs=2))
            ps_mm = ph12.enter_context(tc.tile_pool(name="ps_mm", bufs=4, space="PSUM"))

            # ---- project own key half: V-own and K^T-own -> cc_in ----
            xT = xt_pool.tile([128, 8, 8, 128], F32R, tag="xT", name="xh")
            for lt in range(8):
                nc.gpsimd.dma_start(out=xT[:, lt, :, :], in_=x_h[lt][:, :, :])
            for eh in range(2):
                wv_sb = wv_pool.tile([128, 8, 512], F32R, tag="wv", name=f"wv{eh}")
                for h2 in range(2):
                    nc.scalar.dma_start(
                        out=wv_sb[:, h2 * 4:(h2 + 1) * 4, :],
                        in_=wv[eh][:, h2 * 4:(h2 + 1) * 4, :],
                    )
                for lt in range(8):
                    vps = ps_mm.tile([128, 512], F32, tag="mm", name=f"v{eh}_{lt}")
                    for c in range(8):
                        nc.tensor.matmul(
                            vps, xT[:, lt, c, :], wv_sb[:, c, :],
                            start=(c == 0), stop=(c == 7),
                        )
                    vst = qst_pool.tile([128, 512], F32R, tag="qs", name=f"vs{eh}_{lt}")
                    nc.vector.tensor_copy(vst, vps)
                    nc.sync.dma_start(
                        out=cc_in[:, 8 + lt, eh * 512:(eh + 1) * 512], in_=vst
                    )
            for e in range(8):
                wk_sb = we_pool.tile([128, 8, 128], F32R, tag="we", name=f"wk{e}")
                nc.scalar.dma_start(out=wk_sb, in_=wk[e][:, :, :])
                kps = [ps_mm.tile([128, 512], F32, tag="mm", name=f"k{e}_{g}")
                       for g in range(2)]
                for c in range(8):
                    for kg in range(2):
                        nc.tensor.matmul(
                            kps[kg], wk_sb[:, c, :], xT[:, kg * 4:(kg + 1) * 4, c, :],
                            start=(c == 0), stop=(c == 7),
                        )
                for kg in range(2):
                    kst = qst_pool.tile([128, 512], F32R, tag="qs", name=f"ks{e}_{kg}")
                    nc.vector.tensor_copy(kst, kps[kg])
                    nc.sync.dma_start(
                        out=cc_in[:, e, kg * 512:(kg + 1) * 512], in_=kst
                    )

            # ---- Q^T for own query tiles, spilled to DRAM ----
            xq = xt_pool.tile([128, 8, 8, 128], F32R, tag="xT", name="xq")
            for s in range(N_SLOTS):
                nc.gpsimd.dma_start(out=xq[:, s, :, :], in_=x_qt[s][:, :, :])
            for e in range(8):
                wq_sb = we_pool.tile([128, 8, 128], F32R, tag="we", name=f"wq{e}")
                nc.scalar.dma_start(out=wq_sb, in_=wq[e][:, :, :])
                qps = [ps_mm.tile([128, 512], F32, tag="mm", name=f"q{e}_{g}")
                       for g in range(2)]
                for c in range(8):
                    for qg in range(2):
                        nc.tensor.matmul(
                            qps[qg], wq_sb[:, c, :], xq[:, qg * 4:(qg + 1) * 4, c, :],
                            start=(c == 0), stop=(c == 7),
                        )
                qstage = qst_pool.tile([128, 1024], F32R, tag="qs", name=f"qs{e}")
                for qg in range(2):
                    nc.vector.tensor_copy(qstage[:, qg * 512:(qg + 1) * 512], qps[qg])
                nc.sync.dma_start(
                    out=qt_spill[:, :, e, :].rearrange("s p q -> p s q"),
                    in_=qstage.rearrange("p (s q) -> p s q", s=8),
                )

            # ---- exchange K^T/V halves within the core pair ----
            nc.gpsimd.collective_compute(
                "AllGather",
                mybir.AluOpType.bypass,
                replica_groups=[[0, 1], [2, 3], [4, 5], [6, 7]],
                ins=[cc_in.opt()],
                outs=[cc_out.opt()],
            )
            for h in range(2):
                nc.gpsimd.dma_start(
                    out=KT[:, :, h * 1024:(h + 1) * 1024],
                    in_=cc_out[h][:, 0:8, :],
                )
            nc.gpsimd.dma_start(out=V[:, 0:8, :], in_=cc_out[0][:, 8:16, :])
            nc.gpsimd.dma_start(out=V[:, 8:12, :], in_=cc_out[1][:, 8:12, :])

        # ---- attention slots 0-7, software-pipelined AV ----
        with ExitStack() as ph3:
            p_hi = ph3.enter_context(tc.tile_pool(name="phi", bufs=2))
            pt_pool = ph3.enter_context(tc.tile_pool(name="ptp", bufs=2))
            sc_pool = ph3.enter_context(tc.tile_pool(name="scp", bufs=2))
            outp = ph3.enter_context(tc.tile_pool(name="outp", bufs=2))
            vh_pool = ph3.enter_context(tc.tile_pool(name="vhp", bufs=1))
            v_hi = {}

            def emit_av(i, L, P_sb, recip):
                O_ps = ps_o.tile([128, D], F32, tag="O", name=f"O{i}")
                for kt in range(L):
                    ptps = ps_tr.tile([128, 128], F32R, tag="tr", name=f"tp{i}_{kt}")
                    nc.tensor.transpose(ptps, P_sb[:, kt * 128:(kt + 1) * 128], ident)
                    pt_sb = pt_pool.tile([128, 128], F32R, tag="pts", name=f"pt{i}_{kt}")
                    nc.vector.tensor_copy(pt_sb, ptps)
                    vsrc = V[:, kt, :] if kt < 12 else v_hi[kt - 12]
                    for h in range(2):
                        nc.tensor.matmul(
                            O_ps[:, h * 512:(h + 1) * 512], pt_sb,
                            vsrc[:, h * 512:(h + 1) * 512],
                            start=(kt == 0), stop=(kt == L - 1),
                        )
                out_sb = outp.tile([128, D], F32, tag="osb", name=f"ou{i}")
                nc.vector.tensor_scalar_mul(out_sb, O_ps, recip)
                nc.sync.dma_start(out=out_q[i][:, :], in_=out_sb)

            def do_slot(i, ps_pool, s_width, prev):
                L = 2 * (i + 1)
                qt_sb = qt_pool2.tile([128, 8, 128], F32R, tag="qt", name=f"qt{i}")
                nc.sync.dma_start(out=qt_sb, in_=qt_spill[i][:, :, :])
                S_ps = ps_pool.tile([128, s_width], F32, tag="S", name=f"S{i}")
                ngroups = (L * 128 + 511) // 512
                for e in range(8):
                    for kg in range(ngroups):
                        w = min(512, L * 128 - kg * 512)
                        nc.tensor.matmul(
                            S_ps[:, kg * 512: kg * 512 + w],
                            qt_sb[:, e, :],
                            KT[:, e, kg * 512: kg * 512 + w],
                            start=(e == 0), stop=(e == 7),
                        )
                # scores/32 are bounded (|s|/32 <~ 11) -> exp without max-subtraction
                nc.vector.tensor_add(
                    S_ps[:, (L - 2) * 128: L * 128],
                    S_ps[:, (L - 2) * 128: L * 128],
                    mask_sb,
                )
                P_sb = p_hi.tile([128, N], F32R, tag="P", name=f"P{i}")
                stats = sc_pool.tile([128, 4], F32, tag="stats", name=f"st{i}")
                rowsum = stats[:, 2:3]
                nc.scalar.activation(
                    P_sb[:, : L * 128], S_ps[:, : L * 128],
                    mybir.ActivationFunctionType.Exp,
                    bias=0.0, scale=SCALE, accum_out=rowsum,
                )
                recip = stats[:, 3:4]
                nc.vector.reciprocal(recip, rowsum)
                if prev is not None:
                    emit_av(*prev)
                return (i, L, P_sb, recip)

            prev = None
            with tc.tile_pool(name="ps_sA", bufs=2, space="PSUM") as ps_sA:
                for i in range(4):
                    prev = do_slot(i, ps_sA, 1024, prev)
            with tc.tile_pool(name="ps_sB", bufs=1, space="PSUM") as ps_sB:
                for i in range(4, 6):
                    prev = do_slot(i, ps_sB, 2048, prev)
                # V high tiles straight from the gathered buffer, before slot 6
                for j in range(4):
                    vh = vh_pool.tile([128, D], F32R, tag=f"vh{j}", name=f"vh{j}")
                    nc.sync.dma_start(out=vh, in_=cc_out[1][:, 12 + j, :])
                    v_hi[j] = vh
                for i in range(6, N_SLOTS):
                    prev = do_slot(i, ps_sB, 2048, prev)
                emit_av(*prev)

    nc.compile()
    return nc


def _masks():
    q = np.arange(128)[:, None]
    k = np.arange(128)[None, :]
    tril_add = np.where(k <= q, 0.0, NEG).astype(np.float32)
    m0 = np.concatenate([tril_add, np.full((128, 128), NEG, np.float32)], axis=1)
    m1 = np.concatenate([np.zeros((128, 128), np.float32), tril_add], axis=1)
    return m0, m1


def kernel(x, Wq, Wk, Wv):
    global LAST_EXEC_NS
    x = np.ascontiguousarray(np.asarray(x, dtype=np.float32))
    Wq = np.ascontiguousarray(np.asarray(Wq, dtype=np.float32))
    Wk = np.ascontiguousarray(np.asarray(Wk, dtype=np.float32))
    Wv = np.ascontiguousarray(np.asarray(Wv, dtype=np.float32))

    if "nc" not in _NC_CACHE:
        _NC_CACHE["nc"] = _build_nc()
    nc = _NC_CACHE["nc"]

    # host pre-transpose: x[b] (N, D) -> (tile, p=d%128, dchunk, token)
    # element (t, p, c, q) = x[b, t*128+q, c*128+p]
    xt_all = np.ascontiguousarray(
        x.reshape(B, N_KTILES, 128, 8, 128).transpose(0, 1, 4, 3, 2)
    )  # [B, tile, p, c, q]

    # weights host-rearranged to give contiguous per-partition DMA runs
    wq_r = np.ascontiguousarray(Wq.reshape(8, 128, 8, 128).transpose(2, 1, 0, 3))
    wk_r = np.ascontiguousarray(Wk.reshape(8, 128, 8, 128).transpose(2, 1, 0, 3))
    wv_r = np.ascontiguousarray(Wv.reshape(8, 128, 2, 512).transpose(2, 1, 0, 3))

    m0, m1 = _masks()
    in_maps = []
    for c in range(N_CORES):
        b, par = divmod(c, 2)
        in_maps.append({
            "x_h": np.ascontiguousarray(xt_all[b, par * 8:(par + 1) * 8]),
            "x_qt": np.ascontiguousarray(xt_all[b, par::2]),
            "wq": wq_r, "wk": wk_r, "wv": wv_r,
            "mask": m1 if par else m0,
        })

    res = run_bass_kernel_spmd(nc, in_maps, list(range(N_CORES)), trace=TRACE)
    LAST_EXEC_NS = res.exec_time_ns

    out = np.empty((B, N, D), dtype=np.float32)
    for c in range(N_CORES):
        b, par = divmod(c, 2)
        oq = res.results[c]["out_q"]
        for i in range(N_SLOTS):
            g = 2 * i + par
            out[b, g * 128:(g + 1) * 128, :] = oq[i]
    return out


# revision 28
# speedup vs baseline: 1.3629x; 1.3629x over previous
"""Causal attention (B=4, N=2048, D=1024) on 8 Trainium2 NeuronCores.

Sharding: core 2b+p handles batch b with query tiles {p, p+2, ..., p+14}
(128-row tiles, parity-interleaved).  Every core runs the same program:
8 query slots with key-tile limits (2, 4, ..., 16) — an exactly balanced
causal split.  Per-core masks are passed as input data so the program is
uniform across cores (SPMD).

All matmuls run in float32r (TF32-like, full PE rate at N>=256); fp32
arrays are fed bit-identically into float32r DRAM params (HW rounds at
the PE input).  x is pre-transposed on the host into d-major tile layout
so no on-chip transposes are needed for the projections.

Schedule: Q^T is computed first and spilled to DRAM; then keys are
processed in two halves (V + K^T into SBUF-resident tiles), with
attention slots 0-3 placed between the halves so the scheduler can
overlap early attention with the second half's projections.  Softmax is
single-pass over the full key row (<= 4 PSUM banks) with exp + row-sum
fused on the scalar engine.
"""
import sys

sys.path.insert(0, "/opt/trn_rl_repo")

from contextlib import ExitStack

import numpy as np

import concourse.bass as bass
import concourse.mybir as mybir
import concourse.tile as tile
from concourse import bacc
from concourse.bass_utils import run_bass_kernel_spmd
from concourse.masks import make_identity

B, N, D = 4, 2048, 1024
N_CORES = 8
N_SLOTS = 8          # query tiles per core
N_KTILES = 16        # 128-key tiles per batch
SCALE = 1.0 / 32.0   # 1/sqrt(D)
NEG = -1.0e9

F32 = mybir.dt.float32
F32R = mybir.dt.float32r

_NC_CACHE = {}
TRACE = False
LAST_EXEC_NS = None


def _build_nc():
    nc = bacc.Bacc(None, target_bir_lowering=False, debug=False)

    # x pre-transposed on host: [tile, partition(d%128), dchunk, token]
    x_t = nc.declare_dram_parameter("x_t", [N_KTILES, 128, 8, 128], F32R, isOutput=False)
    x_qt = nc.declare_dram_parameter("x_qt", [N_SLOTS, 128, 8, 128], F32R, isOutput=False)
    # weights host-rearranged: wq/wk [echunk, p(d%128), dchunk, ecol]; wv [eh, p, dchunk, ecol]
    wq = nc.declare_dram_parameter("wq", [8, 128, 8, 128], F32R, isOutput=False)
    wk = nc.declare_dram_parameter("wk", [8, 128, 8, 128], F32R, isOutput=False)
    wv = nc.declare_dram_parameter("wv", [2, 128, 8, 512], F32R, isOutput=False)
    mask_in = nc.declare_dram_parameter("mask", [128, 256], F32, isOutput=False)
    out_q = nc.declare_dram_parameter("out_q", [N_SLOTS, 128, D], F32, isOutput=True)

    # DRAM scratch: Q^T per-slot-contiguous, V spill for key tiles 13..15
    qt_spill = nc.dram_tensor("qt_spill", [N_SLOTS, 128, 8, 128], F32R, kind="Internal")
    v_spill = nc.dram_tensor("v_spill", [2, 128, D], F32R, kind="Internal")

    with tile.TileContext(nc) as tc, ExitStack() as top:
        consts = top.enter_context(tc.tile_pool(name="consts", bufs=1))
        kt_pool = top.enter_context(tc.tile_pool(name="ktp", bufs=1))
        v_pool = top.enter_context(tc.tile_pool(name="vp", bufs=1))
        qt_pool2 = top.enter_context(tc.tile_pool(name="qtl", bufs=2))
        ps_tr = top.enter_context(tc.tile_pool(name="ps_tr", bufs=2, space="PSUM"))
        ps_o = top.enter_context(tc.tile_pool(name="ps_o", bufs=1, space="PSUM"))

        ident_f = consts.tile([128, 128], F32)
        make_identity(nc, ident_f)
        ident = consts.tile([128, 128], F32R)
        nc.vector.tensor_copy(ident, ident_f)
        mask_sb = consts.tile([128, 256], F32)
        nc.sync.dma_start(out=mask_sb, in_=mask_in[:, :])

        KT = kt_pool.tile([128, 8, N], F32R)      # [p(e%128), echunk, key]
        V = v_pool.tile([128, 14, D], F32R)

        with ExitStack() as ph12:
            xt_pool = ph12.enter_context(tc.tile_pool(name="xtp", bufs=1))
            wv_pool = ph12.enter_context(tc.tile_pool(name="wvp", bufs=2))
            we_pool = ph12.enter_context(tc.tile_pool(name="wep", bufs=2))
            qst_pool = ph12.enter_context(tc.tile_pool(name="qst", bufs=1))
            ps_mm = ph12.enter_context(tc.tile_pool(name="ps_mm", bufs=4, space="PSUM"))

            def project_keys(kh):
                """V and K^T for key tiles kh*8 .. kh*8+7."""
                xT = xt_pool.tile([128, 8, 8, 128], F32R, tag="xT", name=f"xk{kh}")
                for lt in range(8):
                    t = kh * 8 + lt
                    nc.gpsimd.dma_start(out=xT[:, lt, :, :], in_=x_t[t][:, :, :])
                for eh in range(2):
                    wv_sb = wv_pool.tile([128, 8, 512], F32R, tag="wv", name=f"wv{kh}_{eh}")
                    for h2 in range(2):
                        nc.scalar.dma_start(
                            out=wv_sb[:, h2 * 4:(h2 + 1) * 4, :],
                            in_=wv[eh][:, h2 * 4:(h2 + 1) * 4, :],
                        )
                    for lt in range(8):
                        t = kh * 8 + lt
                        vps = ps_mm.tile([128, 512], F32, tag="mm", name=f"v{kh}_{eh}_{lt}")
                        for c in range(8):
                            nc.tensor.matmul(
                                vps, xT[:, lt, c, :], wv_sb[:, c, :],
                                start=(c == 0), stop=(c == 7),
                            )
                        if t < 14:
                            nc.vector.tensor_copy(V[:, t, eh * 512:(eh + 1) * 512], vps)
                        else:
                            vst = qst_pool.tile([128, 512], F32R, tag="qs", name=f"vs{t}_{eh}")
                            nc.vector.tensor_copy(vst, vps)
                            nc.sync.dma_start(
                                out=v_spill[t - 14][:, eh * 512:(eh + 1) * 512], in_=vst
                            )
                for e in range(8):
                    wk_sb = we_pool.tile([128, 8, 128], F32R, tag="we", name=f"wk{kh}_{e}")
                    nc.scalar.dma_start(out=wk_sb, in_=wk[e][:, :, :])
                    kps = [ps_mm.tile([128, 512], F32, tag="mm", name=f"k{kh}_{e}_{g}")
                           for g in range(2)]
                    for c in range(8):
                        for kg in range(2):
                            nc.tensor.matmul(
                                kps[kg], wk_sb[:, c, :], xT[:, kg * 4:(kg + 1) * 4, c, :],
                                start=(c == 0), stop=(c == 7),
                            )
                    for kg in range(2):
                        nc.vector.tensor_copy(
                            KT[:, e, (kh * 2 + kg) * 512:(kh * 2 + kg + 1) * 512], kps[kg]
                        )

            def project_queries():
                xT = xt_pool.tile([128, 8, 8, 128], F32R, tag="xT", name="xq")
                for s in range(N_SLOTS):
                    nc.gpsimd.dma_start(out=xT[:, s, :, :], in_=x_qt[s][:, :, :])
                for e in range(8):
                    wq_sb = we_pool.tile([128, 8, 128], F32R, tag="we", name=f"wq{e}")
                    nc.scalar.dma_start(out=wq_sb, in_=wq[e][:, :, :])
                    qps = [ps_mm.tile([128, 512], F32, tag="mm", name=f"q{e}_{g}")
                           for g in range(2)]
                    for c in range(8):
                        for qg in range(2):
                            nc.tensor.matmul(
                                qps[qg], wq_sb[:, c, :], xT[:, qg * 4:(qg + 1) * 4, c, :],
                                start=(c == 0), stop=(c == 7),
                            )
                    qstage = qst_pool.tile([128, 1024], F32R, tag="qs", name=f"qs{e}")
                    for qg in range(2):
                        nc.vector.tensor_copy(qstage[:, qg * 512:(qg + 1) * 512], qps[qg])
                    nc.sync.dma_start(
                        out=qt_spill[:, :, e, :].rearrange("s p q -> p s q"),
                        in_=qstage.rearrange("p (s q) -> p s q", s=8),
                    )

            project_keys(0)
            project_queries()  # qt spill roundtrip + kh1 x loads hide here
            project_keys(1)

        # ---- attention slots 0-7, software-pipelined AV ----
        with ExitStack() as ph3:
            p_hi = ph3.enter_context(tc.tile_pool(name="phi", bufs=2))
            pt_pool = ph3.enter_context(tc.tile_pool(name="ptp", bufs=2))
            sc_pool = ph3.enter_context(tc.tile_pool(name="scp", bufs=2))
            outp = ph3.enter_context(tc.tile_pool(name="outp", bufs=2))
            vh_pool = ph3.enter_context(tc.tile_pool(name="vhp", bufs=1))
            v_hi = []
            for j in range(2):
                vh = vh_pool.tile([128, D], F32R, tag=f"vh{j}", name=f"vh{j}")
                nc.gpsimd.dma_start(out=vh, in_=v_spill[j][:, :])
                v_hi.append(vh)

            def emit_av(i, L, P_sb, recip):
                O_ps = ps_o.tile([128, D], F32, tag="O", name=f"O{i}")
                for kt in range(L):
                    ptps = ps_tr.tile([128, 128], F32R, tag="tr", name=f"tp{i}_{kt}")
                    nc.tensor.transpose(ptps, P_sb[:, kt * 128:(kt + 1) * 128], ident)
                    pt_sb = pt_pool.tile([128, 128], F32R, tag="pts", name=f"pt{i}_{kt}")
                    nc.vector.tensor_copy(pt_sb, ptps)
                    vsrc = V[:, kt, :] if kt < 14 else v_hi[kt - 14]
                    for h in range(2):
                        nc.tensor.matmul(
                            O_ps[:, h * 512:(h + 1) * 512], pt_sb,
                            vsrc[:, h * 512:(h + 1) * 512],
                            start=(kt == 0), stop=(kt == L - 1),
                        )
                out_sb = outp.tile([128, D], F32, tag="osb", name=f"ou{i}")
                nc.vector.tensor_scalar_mul(out_sb, O_ps, recip)
                nc.sync.dma_start(out=out_q[i][:, :], in_=out_sb)

            def do_slot(i, ps_pool, s_width, prev):
                L = 2 * (i + 1)
                qt_sb = qt_pool2.tile([128, 8, 128], F32R, tag="qt", name=f"qt{i}")
                nc.gpsimd.dma_start(out=qt_sb, in_=qt_spill[i][:, :, :])
                S_ps = ps_pool.tile([128, s_width], F32, tag="S", name=f"S{i}")
                ngroups = (L * 128 + 511) // 512
                for e in range(8):
                    for kg in range(ngroups):
                        w = min(512, L * 128 - kg * 512)
                        nc.tensor.matmul(
                            S_ps[:, kg * 512: kg * 512 + w],
                            qt_sb[:, e, :],
                            KT[:, e, kg * 512: kg * 512 + w],
                            start=(e == 0), stop=(e == 7),
                        )
                # scores/32 are bounded (|s|/32 <~ 11) -> exp without max-subtraction
                nc.vector.tensor_add(
                    S_ps[:, (L - 2) * 128: L * 128],
                    S_ps[:, (L - 2) * 128: L * 128],
                    mask_sb,
                )
                P_sb = p_hi.tile([128, N], F32R, tag="P", name=f"P{i}")
                stats = sc_pool.tile([128, 4], F32, tag="stats", name=f"st{i}")
                rowsum = stats[:, 2:3]
                nc.scalar.activation(
                    P_sb[:, : L * 128], S_ps[:, : L * 128],
                    mybir.ActivationFunctionType.Exp,
                    bias=0.0, scale=SCALE, accum_out=rowsum,
                )
                recip = stats[:, 3:4]
                nc.vector.reciprocal(recip, rowsum)
                if prev is not None:
                    emit_av(*prev)
                return (i, L, P_sb, recip)

            prev = None
            with tc.tile_pool(name="ps_sA", bufs=2, space="PSUM") as ps_sA:
                for i in range(4):
                    prev = do_slot(i, ps_sA, 1024, prev)
            with tc.tile_pool(name="ps_sB", bufs=1, space="PSUM") as ps_sB:
                for i in range(4, N_SLOTS):
                    prev = do_slot(i, ps_sB, 2048, prev)
                emit_av(*prev)

    nc.compile()
    return nc


def _masks():
    q = np.arange(128)[:, None]
    k = np.arange(128)[None, :]
    tril_add = np.where(k <= q, 0.0, NEG).astype(np.float32)
    m0 = np.concatenate([tril_add, np.full((128, 128), NEG, np.float32)], axis=1)
    m1 = np.concatenate([np.zeros((128, 128), np.float32), tril_add], axis=1)
    return m0, m1


def kernel(x, Wq, Wk, Wv):
    global LAST_EXEC_NS
    x = np.ascontiguousarray(np.asarray(x, dtype=np.float32))
    Wq = np.ascontiguousarray(np.asarray(Wq, dtype=np.float32))
    Wk = np.ascontiguousarray(np.asarray(Wk, dtype=np.float32))
    Wv = np.ascontiguousarray(np.asarray(Wv, dtype=np.float32))

    if "nc" not in _NC_CACHE:
        _NC_CACHE["nc"] = _build_nc()
    nc = _NC_CACHE["nc"]

    # host pre-transpose: x[b] (N, D) -> (tile, p=d%128, dchunk, token)
    # element (t, p, c, q) = x[b, t*128+q, c*128+p]
    xt_all = np.ascontiguousarray(
        x.reshape(B, N_KTILES, 128, 8, 128).transpose(0, 1, 4, 3, 2)
    )  # [B, tile, p, c, q]

    # weights host-rearranged to give contiguous per-partition DMA runs
    wq_r = np.ascontiguousarray(Wq.reshape(8, 128, 8, 128).transpose(2, 1, 0, 3))
    wk_r = np.ascontiguousarray(Wk.reshape(8, 128, 8, 128).transpose(2, 1, 0, 3))
    wv_r = np.ascontiguousarray(Wv.reshape(8, 128, 2, 512).transpose(2, 1, 0, 3))

    m0, m1 = _masks()
    in_maps = []
    for c in range(N_CORES):
        b, par = divmod(c, 2)
        in_maps.append({
            "x_t": xt_all[b],
            "x_qt": np.ascontiguousarray(xt_all[b, par::2]),
            "wq": wq_r, "wk": wk_r, "wv": wv_r,
            "mask": m1 if par else m0,
        })

    res = run_bass_kernel_spmd(nc, in_maps, list(range(N_CORES)), trace=TRACE)
    LAST_EXEC_NS = res.exec_time_ns

    out = np.empty((B, N, D), dtype=np.float32)
    for c in range(N_CORES):
        b, par = divmod(c, 2)
        oq = res.results[c]["out_q"]
        for i in range(N_SLOTS):
            g = 2 * i + par
            out[b, g * 128:(g + 1) * 128, :] = oq[i]
    return out


# revision 29
# speedup vs baseline: 1.4651x; 1.0750x over previous
"""Causal attention (B=4, N=2048, D=1024) on 8 Trainium2 NeuronCores.

Sharding: core 2b+p handles batch b with query tiles {p, p+2, ..., p+14}
(128-row tiles, parity-interleaved).  Every core runs the same program:
8 query slots with key-tile limits (2, 4, ..., 16) — an exactly balanced
causal split.  Per-core masks are passed as input data so the program is
uniform across cores (SPMD).

All matmuls run in float32r (TF32-like, full PE rate at N>=256); fp32
arrays are fed bit-identically into float32r DRAM params (HW rounds at
the PE input).  x is pre-transposed on the host into d-major tile layout
so no on-chip transposes are needed for the projections.

Schedule: Q^T is computed first and spilled to DRAM; then keys are
processed in two halves (V + K^T into SBUF-resident tiles), with
attention slots 0-3 placed between the halves so the scheduler can
overlap early attention with the second half's projections.  Softmax is
single-pass over the full key row (<= 4 PSUM banks) with exp + row-sum
fused on the scalar engine.
"""
import sys

sys.path.insert(0, "/opt/trn_rl_repo")

from contextlib import ExitStack

import numpy as np

import concourse.bass as bass
import concourse.mybir as mybir
import concourse.tile as tile
from concourse import bacc
from concourse.bass_utils import run_bass_kernel_spmd
from concourse.masks import make_identity

B, N, D = 4, 2048, 1024
N_CORES = 8
N_SLOTS = 8          # query tiles per core
N_KTILES = 16        # 128-key tiles per batch
SCALE = 1.0 / 32.0   # 1/sqrt(D)
NEG = -1.0e9

F32 = mybir.dt.float32
F32R = mybir.dt.float32r

_NC_CACHE = {}
TRACE = False
LAST_EXEC_NS = None


def _build_nc():
    nc = bacc.Bacc(None, target_bir_lowering=False, debug=False)

    # x pre-transposed on host: [tile, partition(d%128), dchunk, token]
    x_t = nc.declare_dram_parameter("x_t", [N_KTILES, 128, 8, 128], F32R, isOutput=False)
    x_qt = nc.declare_dram_parameter("x_qt", [N_SLOTS, 128, 8, 128], F32R, isOutput=False)
    # weights host-rearranged: wq/wk [echunk, p(d%128), dchunk, ecol]; wv [eh, p, dchunk, ecol]
    wq = nc.declare_dram_parameter("wq", [8, 128, 8, 128], F32R, isOutput=False)
    wk = nc.declare_dram_parameter("wk", [8, 128, 8, 128], F32R, isOutput=False)
    wv = nc.declare_dram_parameter("wv", [2, 128, 8, 512], F32R, isOutput=False)
    mask_in = nc.declare_dram_parameter("mask", [128, 256], F32, isOutput=False)
    out_q = nc.declare_dram_parameter("out_q", [N_SLOTS, 128, D], F32, isOutput=True)

    # DRAM scratch: Q^T per-slot-contiguous, V spill for key tiles 13..15
    qt_spill = nc.dram_tensor("qt_spill", [N_SLOTS, 128, 8, 128], F32R, kind="Internal")
    v_spill = nc.dram_tensor("v_spill", [2, 128, D], F32R, kind="Internal")

    with tile.TileContext(nc) as tc, ExitStack() as top:
        consts = top.enter_context(tc.tile_pool(name="consts", bufs=1))
        kt_pool = top.enter_context(tc.tile_pool(name="ktp", bufs=1))
        v_pool = top.enter_context(tc.tile_pool(name="vp", bufs=1))
        qt_pool2 = top.enter_context(tc.tile_pool(name="qtl", bufs=2))
        ps_tr = top.enter_context(tc.tile_pool(name="ps_tr", bufs=2, space="PSUM"))
        ps_o = top.enter_context(tc.tile_pool(name="ps_o", bufs=1, space="PSUM"))

        ident_f = consts.tile([128, 128], F32)
        make_identity(nc, ident_f)
        ident = consts.tile([128, 128], F32R)
        nc.vector.tensor_copy(ident, ident_f)
        mask_sb = consts.tile([128, 256], F32)
        nc.sync.dma_start(out=mask_sb, in_=mask_in[:, :])

        KT = kt_pool.tile([128, 8, N], F32R)      # [p(e%128), echunk, key]
        V = v_pool.tile([128, 14, D], F32R)

        with ExitStack() as ph12:
            xt_pool = ph12.enter_context(tc.tile_pool(name="xtp", bufs=1))
            wv_pool = ph12.enter_context(tc.tile_pool(name="wvp", bufs=2))
            we_pool = ph12.enter_context(tc.tile_pool(name="wep", bufs=2))
            qst_pool = ph12.enter_context(tc.tile_pool(name="qst", bufs=1))
            ps_mm = ph12.enter_context(tc.tile_pool(name="ps_mm", bufs=4, space="PSUM"))

            def project_keys(kh):
                """V and K^T for key tiles kh*8 .. kh*8+7."""
                xT = xt_pool.tile([128, 8, 8, 128], F32R, tag="xT", name=f"xk{kh}")
                for lt in range(8):
                    t = kh * 8 + lt
                    nc.gpsimd.dma_start(out=xT[:, lt, :, :], in_=x_t[t][:, :, :])
                for eh in range(2):
                    wv_sb = wv_pool.tile([128, 8, 512], F32R, tag="wv", name=f"wv{kh}_{eh}")
                    for h2 in range(2):
                        nc.scalar.dma_start(
                            out=wv_sb[:, h2 * 4:(h2 + 1) * 4, :],
                            in_=wv[eh][:, h2 * 4:(h2 + 1) * 4, :],
                        )
                    for lt in range(8):
                        t = kh * 8 + lt
                        vps = ps_mm.tile([128, 512], F32, tag="mm", name=f"v{kh}_{eh}_{lt}")
                        for c in range(8):
                            nc.tensor.matmul(
                                vps, xT[:, lt, c, :], wv_sb[:, c, :],
                                start=(c == 0), stop=(c == 7),
                            )
                        if t < 14:
                            nc.vector.tensor_copy(V[:, t, eh * 512:(eh + 1) * 512], vps)
                        else:
                            vst = qst_pool.tile([128, 512], F32R, tag="qs", name=f"vs{t}_{eh}")
                            nc.vector.tensor_copy(vst, vps)
                            nc.sync.dma_start(
                                out=v_spill[t - 14][:, eh * 512:(eh + 1) * 512], in_=vst
                            )
                for e in range(8):
                    wk_sb = we_pool.tile([128, 8, 128], F32R, tag="we", name=f"wk{kh}_{e}")
                    nc.scalar.dma_start(out=wk_sb, in_=wk[e][:, :, :])
                    kps = [ps_mm.tile([128, 512], F32, tag="mm", name=f"k{kh}_{e}_{g}")
                           for g in range(2)]
                    for c in range(8):
                        for kg in range(2):
                            nc.tensor.matmul(
                                kps[kg], wk_sb[:, c, :], xT[:, kg * 4:(kg + 1) * 4, c, :],
                                start=(c == 0), stop=(c == 7),
                            )
                    for kg in range(2):
                        nc.vector.tensor_copy(
                            KT[:, e, (kh * 2 + kg) * 512:(kh * 2 + kg + 1) * 512], kps[kg]
                        )

            def project_queries():
                xT = xt_pool.tile([128, 8, 8, 128], F32R, tag="xT", name="xq")
                for s in range(N_SLOTS):
                    nc.gpsimd.dma_start(out=xT[:, s, :, :], in_=x_qt[s][:, :, :])
                for e in range(8):
                    wq_sb = we_pool.tile([128, 8, 128], F32R, tag="we", name=f"wq{e}")
                    nc.scalar.dma_start(out=wq_sb, in_=wq[e][:, :, :])
                    qps = [ps_mm.tile([128, 512], F32, tag="mm", name=f"q{e}_{g}")
                           for g in range(2)]
                    for c in range(8):
                        for qg in range(2):
                            nc.tensor.matmul(
                                qps[qg], wq_sb[:, c, :], xT[:, qg * 4:(qg + 1) * 4, c, :],
                                start=(c == 0), stop=(c == 7),
                            )
                    qstage = qst_pool.tile([128, 1024], F32R, tag="qs", name=f"qs{e}")
                    for qg in range(2):
                        nc.vector.tensor_copy(qstage[:, qg * 512:(qg + 1) * 512], qps[qg])
                    nc.sync.dma_start(
                        out=qt_spill[:, :, e, :].rearrange("s p q -> p s q"),
                        in_=qstage.rearrange("p (s q) -> p s q", s=8),
                    )

            project_keys(0)
            project_queries()  # qt spill roundtrip + kh1 x loads hide here
            project_keys(1)

        # ---- attention slots 0-7, software-pipelined AV ----
        with ExitStack() as ph3:
            p_hi = ph3.enter_context(tc.tile_pool(name="phi", bufs=2))
            pt_pool = ph3.enter_context(tc.tile_pool(name="ptp", bufs=2))
            sc_pool = ph3.enter_context(tc.tile_pool(name="scp", bufs=2))
            outp = ph3.enter_context(tc.tile_pool(name="outp", bufs=2))
            vh_pool = ph3.enter_context(tc.tile_pool(name="vhp", bufs=1))
            v_hi = []

            def emit_av(i, L, P_sb, recip):
                O_ps = ps_o.tile([128, D], F32, tag="O", name=f"O{i}")
                for kt in range(L):
                    ptps = ps_tr.tile([128, 128], F32R, tag="tr", name=f"tp{i}_{kt}")
                    nc.tensor.transpose(ptps, P_sb[:, kt * 128:(kt + 1) * 128], ident)
                    pt_sb = pt_pool.tile([128, 128], F32R, tag="pts", name=f"pt{i}_{kt}")
                    nc.vector.tensor_copy(pt_sb, ptps)
                    vsrc = V[:, kt, :] if kt < 14 else v_hi[kt - 14]
                    for h in range(2):
                        nc.tensor.matmul(
                            O_ps[:, h * 512:(h + 1) * 512], pt_sb,
                            vsrc[:, h * 512:(h + 1) * 512],
                            start=(kt == 0), stop=(kt == L - 1),
                        )
                out_sb = outp.tile([128, D], F32, tag="osb", name=f"ou{i}")
                nc.vector.tensor_scalar_mul(out_sb, O_ps, recip)
                nc.sync.dma_start(out=out_q[i][:, :], in_=out_sb)

            def do_slot(i, ps_pool, s_width, prev):
                L = 2 * (i + 1)
                qt_sb = qt_pool2.tile([128, 8, 128], F32R, tag="qt", name=f"qt{i}")
                nc.gpsimd.dma_start(out=qt_sb, in_=qt_spill[i][:, :, :])
                S_ps = ps_pool.tile([128, s_width], F32, tag="S", name=f"S{i}")
                ngroups = (L * 128 + 511) // 512
                for e in range(8):
                    for kg in range(ngroups):
                        w = min(512, L * 128 - kg * 512)
                        nc.tensor.matmul(
                            S_ps[:, kg * 512: kg * 512 + w],
                            qt_sb[:, e, :],
                            KT[:, e, kg * 512: kg * 512 + w],
                            start=(e == 0), stop=(e == 7),
                        )
                # scores/32 are bounded (|s|/32 <~ 11) -> exp without max-subtraction
                nc.vector.tensor_add(
                    S_ps[:, (L - 2) * 128: L * 128],
                    S_ps[:, (L - 2) * 128: L * 128],
                    mask_sb,
                )
                P_sb = p_hi.tile([128, N], F32R, tag="P", name=f"P{i}")
                stats = sc_pool.tile([128, 4], F32, tag="stats", name=f"st{i}")
                rowsum = stats[:, 2:3]
                nc.scalar.activation(
                    P_sb[:, : L * 128], S_ps[:, : L * 128],
                    mybir.ActivationFunctionType.Exp,
                    bias=0.0, scale=SCALE, accum_out=rowsum,
                )
                recip = stats[:, 3:4]
                nc.vector.reciprocal(recip, rowsum)
                if prev is not None:
                    emit_av(*prev)
                return (i, L, P_sb, recip)

            prev = None
            with tc.tile_pool(name="ps_sA", bufs=2, space="PSUM") as ps_sA:
                for i in range(4):
                    prev = do_slot(i, ps_sA, 1024, prev)
            with tc.tile_pool(name="ps_sB", bufs=1, space="PSUM") as ps_sB:
                for i in range(4, 6):
                    prev = do_slot(i, ps_sB, 2048, prev)
                for j in range(2):
                    vh = vh_pool.tile([128, D], F32R, tag=f"vh{j}", name=f"vh{j}")
                    nc.sync.dma_start(out=vh, in_=v_spill[j][:, :])
                    v_hi.append(vh)
                for i in range(6, N_SLOTS):
                    prev = do_slot(i, ps_sB, 2048, prev)
                emit_av(*prev)

    nc.compile()
    return nc


def _masks():
    q = np.arange(128)[:, None]
    k = np.arange(128)[None, :]
    tril_add = np.where(k <= q, 0.0, NEG).astype(np.float32)
    m0 = np.concatenate([tril_add, np.full((128, 128), NEG, np.float32)], axis=1)
    m1 = np.concatenate([np.zeros((128, 128), np.float32), tril_add], axis=1)
    return m0, m1


def kernel(x, Wq, Wk, Wv):
    global LAST_EXEC_NS
    x = np.ascontiguousarray(np.asarray(x, dtype=np.float32))
    Wq = np.ascontiguousarray(np.asarray(Wq, dtype=np.float32))
    Wk = np.ascontiguousarray(np.asarray(Wk, dtype=np.float32))
    Wv = np.ascontiguousarray(np.asarray(Wv, dtype=np.float32))

    if "nc" not in _NC_CACHE:
        _NC_CACHE["nc"] = _build_nc()
    nc = _NC_CACHE["nc"]

    # host pre-transpose: x[b] (N, D) -> (tile, p=d%128, dchunk, token)
    # element (t, p, c, q) = x[b, t*128+q, c*128+p]
    xt_all = np.ascontiguousarray(
        x.reshape(B, N_KTILES, 128, 8, 128).transpose(0, 1, 4, 3, 2)
    )  # [B, tile, p, c, q]

    # weights host-rearranged to give contiguous per-partition DMA runs
    wq_r = np.ascontiguousarray(Wq.reshape(8, 128, 8, 128).transpose(2, 1, 0, 3))
    wk_r = np.ascontiguousarray(Wk.reshape(8, 128, 8, 128).transpose(2, 1, 0, 3))
    wv_r = np.ascontiguousarray(Wv.reshape(8, 128, 2, 512).transpose(2, 1, 0, 3))

    m0, m1 = _masks()
    in_maps = []
    for c in range(N_CORES):
        b, par = divmod(c, 2)
        in_maps.append({
            "x_t": xt_all[b],
            "x_qt": np.ascontiguousarray(xt_all[b, par::2]),
            "wq": wq_r, "wk": wk_r, "wv": wv_r,
            "mask": m1 if par else m0,
        })

    res = run_bass_kernel_spmd(nc, in_maps, list(range(N_CORES)), trace=TRACE)
    LAST_EXEC_NS = res.exec_time_ns

    out = np.empty((B, N, D), dtype=np.float32)
    for c in range(N_CORES):
        b, par = divmod(c, 2)
        oq = res.results[c]["out_q"]
        for i in range(N_SLOTS):
            g = 2 * i + par
            out[b, g * 128:(g + 1) * 128, :] = oq[i]
    return out
